# revision 1
# baseline (speedup 1.0000x reference)
"""Multi-head attention forward (b=8, n=2048, dim=512, heads=8, dh=64) on 8
Trainium2 NeuronCores.

Sharding: data-parallel over batch — core i computes the full attention layer
for batch element i (weights replicated, no collectives).

Per-core pipeline (everything "transposed" so softmax rowsums come out of the
same matmuls):
  1. x [2048,512] -> PE-transpose -> xT [512,2048]
  2. qkT = w_qk.T @ xT           [1024,2048]  (q/k features on partitions)
  3. v   = x @ w_v               [2048,512]   (tokens on partitions) + ones col
  4. per head h, per pair of 512-wide query blocks:
       simT[j,i]  = kT_h[:,j].T @ qT_h[:,i]      (K=64)
       expT       = exp(SCALE*simT)              (one ACT op per [128,1024])
       outT_aug   = sum_j v_aug[j].T @ expT      (K=128; row 64 = exp rowsum)
       rinv       = 1/rowsum; broadcast over 64 partitions via K=1 matmul
       attn_outT[hd, i] = outT_aug[0:64] * rinv_bcast
  5. out = attn_outT.T @ w_out   [2048,512]

Matmuls run as float32r (full-rate streaming vs 4x slower float32). The
hardware requires f32r matmul operands to be *produced* rounded, so every SBUF
tile feeding a matmul is float32r, written by a rounding op (copy/activation).
Consecutive matmuls share stationary operands where possible (LDWEIGHTS swap
costs ~125ns); ACT ops are batched to [128,1024] to amortize the ~352-cycle
per-instruction overhead of the scalar engine.
"""

import os

import numpy as np

import concourse.bass as bass
import concourse.mybir as mybir
import concourse.tile as tile
from concourse import bacc
from concourse.masks import make_identity

FP32 = mybir.dt.float32
F32R = mybir.dt.float32r

B = 8
N = 2048
D = 512
H = 8
DH = 64
F3 = 3 * D
SCALE = DH**-0.5
P = 128
NT = N // P  # 16 token tiles
CT = D // P  # 4 contraction tiles over dim
NB = N // 512  # 4 query blocks of 512
JT = N // P  # 16 key tiles

_USE_F32R = os.environ.get("BASS_ATTN_FP32", "0") != "1"
MM_DT = F32R if _USE_F32R else FP32

# bisection knobs (timing experiments only; wrong numerics when non-default)
_SKIP_ATTN = os.environ.get("BASS_ATTN_SKIP_ATTN", "0") == "1"
_JT_LIM = int(os.environ.get("BASS_ATTN_JT", str(JT)))
_H_LIM = int(os.environ.get("BASS_ATTN_H", str(H)))


def _attention_body(tc: "tile.TileContext", repeat: int = 1):
    nc = tc.nc
    x = nc.dram_tensor("x", [N, D], FP32, kind="ExternalInput").ap()
    w_qkv = nc.dram_tensor("w_qkv", [D, F3], FP32, kind="ExternalInput").ap()
    w_out = nc.dram_tensor("w_out", [D, D], FP32, kind="ExternalInput").ap()
    out = nc.dram_tensor("out", [N, D], FP32, kind="ExternalOutput").ap()
    for _ in range(repeat):
        _attention_once(tc, x, w_qkv, w_out, out)


def _attention_once(tc: "tile.TileContext", x, w_qkv, w_out, out):
    nc = tc.nc
    exp_f = mybir.ActivationFunctionType.Exp

    with (
        tc.tile_pool(name="const", bufs=1) as const,
        tc.tile_pool(name="persist", bufs=1) as persist,
        tc.tile_pool(name="wstage", bufs=1) as wstage,
    ):
        identity = const.tile([P, P], FP32)
        make_identity(nc, identity)
        # memset can't write f32r; build ones in fp32 and round via copies
        ones32 = const.tile([P, 1], FP32)
        nc.vector.memset(ones32, 1.0)
        ones_1x64 = const.tile([1, 64], MM_DT)
        nc.vector.tensor_copy(out=ones_1x64, in_=ones32[0:1, :].to_broadcast([1, 64]))

        # weights: DMA fp32 -> rounding copy into MM_DT tiles
        wout_sb = persist.tile([P, CT, D], MM_DT)
        for t in range(CT):
            if _USE_F32R:
                ws = wstage.tile([P, F3], FP32, tag="ws")
                nc.sync.dma_start(out=ws[:, :D], in_=w_out[t * P : (t + 1) * P, :])
                nc.vector.tensor_copy(out=wout_sb[:, t, :], in_=ws[:, :D])
            else:
                nc.sync.dma_start(
                    out=wout_sb[:, t, :], in_=w_out[t * P : (t + 1) * P, :]
                )

        # q and k features transposed: rows = 1024 q/k features in 8 tiles
        qkT = persist.tile([P, 8, N], MM_DT)
        # v with tokens on partitions; per head 64 value cols + 1 ones col
        v_aug = persist.tile([P, JT, H * 65], MM_DT)
        nc.vector.tensor_copy(
            out=v_aug.rearrange("p j (h c) -> p j h c", c=65)[:, :, :, 64:65],
            in_=ones32.to_broadcast([P, JT, H, 1]),
        )

        with (
            tc.tile_pool(name="proj", bufs=1) as proj_pool,
            tc.tile_pool(name="xstage", bufs=3) as xstage,
            tc.tile_pool(name="pst", bufs=3, space="PSUM") as pst,
            tc.tile_pool(name="psmm", bufs=2, space="PSUM") as psmm,
        ):
            wqkv_sb = proj_pool.tile([P, CT, F3], MM_DT)
            for t in range(CT):
                if _USE_F32R:
                    ws = wstage.tile([P, F3], FP32, tag="ws")
                    nc.sync.dma_start(out=ws, in_=w_qkv[t * P : (t + 1) * P, :])
                    nc.vector.tensor_copy(out=wqkv_sb[:, t, :], in_=ws)
                else:
                    nc.sync.dma_start(
                        out=wqkv_sb[:, t, :], in_=w_qkv[t * P : (t + 1) * P, :]
                    )

            # ---- load x and transpose to xT [512, 2048] ----
            xT = proj_pool.tile([P, CT, N], MM_DT)
            for j in range(NT):
                xs = xstage.tile([P, D], FP32)
                nc.sync.dma_start(out=xs, in_=x[j * P : (j + 1) * P, :])
                ps = pst.tile([P, CT, P], FP32)  # one bank, 4 transposes
                for t in range(CT):
                    nc.tensor.transpose(
                        ps[:, t, :], xs[:, t * P : (t + 1) * P], identity
                    )
                nc.vector.tensor_copy(out=xT[:, :, j * P : (j + 1) * P], in_=ps)

            # ---- v = x @ w_v (tokens on partitions) ----
            for jp in range(8):
                ps = psmm.tile([P, 2, 512], FP32, tag="mm")
                for c in range(CT):
                    for q in range(2):
                        j = jp * 2 + q
                        nc.tensor.matmul(
                            ps[:, q, :],
                            xT[:, c, j * P : (j + 1) * P],
                            wqkv_sb[:, c, 2 * D : 3 * D],
                            start=(c == 0),
                            stop=(c == CT - 1),
                        )
                nc.vector.tensor_copy(
                    out=v_aug[:, jp * 2 : jp * 2 + 2, :].rearrange(
                        "p j (h c) -> p j h c", c=65
                    )[:, :, :, 0:64],
                    in_=ps.rearrange("p q (h c) -> p q h c", c=64),
                )

            # ---- qkT = w_qk.T @ xT ----
            # m-order pairs each head's q tile with its k tile so head 0's
            # attention can start as early as possible
            for m in (0, 4, 1, 5, 2, 6, 3, 7):
                for nbp in range(2):
                    ps = psmm.tile([P, 2, 512], FP32, tag="mm")
                    for c in range(CT):
                        for q in range(2):
                            nc.tensor.matmul(
                                ps[:, q, :],
                                wqkv_sb[:, c, m * P : (m + 1) * P],
                                xT[:, c, (nbp * 2 + q) * 512 : (nbp * 2 + q + 1) * 512],
                                start=(c == 0),
                                stop=(c == CT - 1),
                            )
                    nc.vector.tensor_copy(
                        out=qkT[:, m, nbp * 1024 : (nbp + 1) * 1024],
                        in_=ps.rearrange("p a b -> p (a b)"),
                    )

        if _SKIP_ATTN:
            with tc.tile_pool(name="skipo", bufs=2) as skipo:
                for jp in range(8):
                    os_ = skipo.tile([P, 2, D], FP32)
                    nc.vector.tensor_copy(out=os_[:, 0, :], in_=qkT[:, jp % 8, 0:512])
                    nc.vector.tensor_copy(
                        out=os_[:, 1, :], in_=v_aug[:, jp, 0:512]
                    )
                    nc.sync.dma_start(
                        out=out[jp * 256 : (jp + 1) * 256, :].rearrange(
                            "(q p) d -> p q d", p=P
                        ),
                        in_=os_,
                    )
            return

        # ---- attention per head, i-blocks in pairs ----
        with (
            tc.tile_pool(name="attno", bufs=1) as attno_pool,
            tc.tile_pool(name="expp", bufs=3) as expp,
            tc.tile_pool(name="rinvp", bufs=2) as rinvp,
            tc.tile_pool(name="outstage", bufs=2) as outstage,
            tc.tile_pool(name="pss", bufs=2, space="PSUM") as pssp,
            tc.tile_pool(name="pso", bufs=2, space="PSUM") as psop,
        ):
            attn_outT = attno_pool.tile([P, CT, N], MM_DT)
            if _H_LIM < H or _JT_LIM < JT:
                nc.vector.tensor_copy(
                    out=attn_outT.rearrange("p c n -> p (c n)")[:, 0:8192],
                    in_=qkT.rearrange("p c n -> p (c n)")[:, 0:8192],
                )
            for h in range(_H_LIM):
                qt, qo = h // 2, (h % 2) * 64
                kt = 4 + h // 2
                for p2 in range(2):
                    pso = psop.tile([P, 2, 512], FP32)
                    # software-pipelined: attnv for j lags one iteration so
                    # the PE never sits behind exp_j in its own stream
                    pending = None
                    for j in range(_JT_LIM):
                        pss = pssp.tile([P, 2, 512], FP32, tag="mm")
                        for q in range(2):
                            ib = p2 * 2 + q
                            nc.tensor.matmul(
                                pss[:, q, :],
                                qkT[qo : qo + 64, kt, j * P : (j + 1) * P],
                                qkT[qo : qo + 64, qt, ib * 512 : (ib + 1) * 512],
                                start=True,
                                stop=True,
                            )
                        ex = expp.tile([P, 2, 512], MM_DT)
                        nc.scalar.activation(
                            out=ex.rearrange("p a b -> p (a b)"),
                            in_=pss.rearrange("p a b -> p (a b)"),
                            func=exp_f,
                            scale=SCALE,
                        )
                        if pending is not None:
                            pex, pj = pending
                            for q in range(2):
                                nc.tensor.matmul(
                                    pso[0:65, q, :],
                                    v_aug[:, pj, h * 65 : (h + 1) * 65],
                                    pex[:, q, :],
                                    start=(pj == 0),
                                    stop=False,
                                )
                        pending = (ex, j)
                    pex, pj = pending
                    for q in range(2):
                        nc.tensor.matmul(
                            pso[0:65, q, :],
                            v_aug[:, pj, h * 65 : (h + 1) * 65],
                            pex[:, q, :],
                            start=(pj == 0),
                            stop=True,
                        )
                    # normalize both 512-blocks of the pair in one go:
                    # rowsums live in pso row 64; 1/x, broadcast to 64
                    # partitions on the idle gpsimd, one wide multiply
                    rinv = rinvp.tile([1, 2, 512], FP32, tag="rinv")
                    nc.vector.reciprocal(out=rinv, in_=pso[64:65, :, :])
                    rb = rinvp.tile([64, 2, 512], FP32, tag="rb")
                    nc.gpsimd.partition_broadcast(rb, rinv)
                    with nc.allow_low_precision("f32r attn out"):
                        nc.vector.tensor_mul(
                            out=attn_outT[
                                qo : qo + 64, qt, p2 * 1024 : (p2 + 1) * 1024
                            ].rearrange("p (a b) -> p a b", a=2),
                            in0=pso[0:64, :, :],
                            in1=rb,
                        )

            # ---- out = attn_outT.T @ w_out ----
            for jp in range(8):
                ps = pssp.tile([P, 2, 512], FP32, tag="mm")
                for t in range(CT):
                    for q in range(2):
                        j = jp * 2 + q
                        nc.tensor.matmul(
                            ps[:, q, :],
                            attn_outT[:, t, j * P : (j + 1) * P],
                            wout_sb[:, t, :],
                            start=(t == 0),
                            stop=(t == CT - 1),
                        )
                os_ = outstage.tile([P, 2, D], FP32)
                nc.vector.tensor_copy(out=os_, in_=ps)
                nc.sync.dma_start(
                    out=out[jp * 256 : (jp + 1) * 256, :].rearrange(
                        "(q p) d -> p q d", p=P
                    ),
                    in_=os_,
                )


_CACHE: dict = {}


def build_nc(repeat: int = 1) -> "bass.Bass":
    key = ("nc", repeat)
    if key not in _CACHE:
        nc = bacc.Bacc("TRN2", target_bir_lowering=False, debug=False)
        with tile.TileContext(nc) as tc:
            _attention_body(tc, repeat=repeat)
        nc.compile()
        _CACHE[key] = nc
    return _CACHE[key]


def kernel(x: np.ndarray, w_qkv: np.ndarray, w_out: np.ndarray) -> np.ndarray:
    from concourse.bass_utils import run_bass_kernel_spmd

    nc = build_nc()
    x = np.ascontiguousarray(np.asarray(x, dtype=np.float32))
    w_qkv = np.ascontiguousarray(np.asarray(w_qkv, dtype=np.float32))
    w_out = np.ascontiguousarray(np.asarray(w_out, dtype=np.float32))
    in_maps = [
        {"x": x[i], "w_qkv": w_qkv, "w_out": w_out} for i in range(B)
    ]
    res = run_bass_kernel_spmd(nc, in_maps, core_ids=list(range(B)))
    return np.stack([r["out"] for r in res.results], axis=0)

